# revision 26
# baseline (speedup 1.0000x reference)
"""Trainium2 Bass kernel for batched GNN message passing.

Computes, for x:[L,G,D], COO edges (rows, cols, vals), W:[D,D], b, gamma, beta:
    xt  = x.transpose(1,0,2).reshape(G, L*D)
    agg = segment_sum(xt[cols] * vals[:,None], rows, G)     # [G, L*D]
    h   = einsum('lgd,od->lgo', agg_as_lgd, W) + b
    s   = silu(h)
    out = layernorm(s) * gamma + beta                        # LN over D

Sharding: nodes (G) are split evenly across 8 NeuronCores; edges are routed
on the host to the core that owns their destination row, grouped into
128-row blocks, and padded to a uniform per-block tile count so all cores
run one SPMD program.  xt is converted to bf16 and replicated into every
core's HBM (split into two <32768-row halves because dma_gather indices are
int16), so message gathering is a local hardware dma_gather (512B rows).

The SWDGE descriptor generation for dma_gather runs on a single Q7
core-pair selected by queue_num (ucode: cpu_id/2 == queue_num), so gathers
are spread round-robin over all 4 SWDGE queues — 4 descriptor generators
run in parallel, which is the main speedup over the 1-queue baseline.

Per 128-edge tile, a one-hot selection matrix S[e,r] = vals[e]*(rowloc[e]==r)
is built on the VectorEngine (bf16) and the segment-sum becomes S.T @ M
accumulated in PSUM.  The 128x128 linear (bf16), SiLU and LayerNorm run
on-chip; PSUM->SBUF copies, the sum/sum-of-squares accumulation, and the
final normalization run on the Scalar engine to keep the VectorEngine free
for the S-tiles.
"""

import numpy as np

L, G, D, E = 2, 50000, 128, 800000
N_CORES = 8
RPC = G // N_CORES            # rows per core = 6250
P = 128
NBLK = (RPC + P - 1) // P     # 49 blocks per core (last block has 106 rows)
F = L * D                     # 256 = packed feature width of xt
NG = N_CORES * NBLK           # 392 (core, block) groups
HALF = 25000                  # xt row-split so gather indices fit in int16
LN_EPS = 1e-5
NQ = 4                        # SWDGE queues used round-robin
NMBUF = 14                    # M gather buffers (3-4 per queue)

_CACHE: dict = {}


def _build_program(T0, T1, apply_bias, apply_gamma, apply_beta):
    import concourse.bacc as bacc
    import concourse.bass as bass
    import concourse.mybir as mybir
    import concourse.tile as tile
    from concourse.masks import make_identity

    f32 = mybir.dt.float32
    bf16 = mybir.dt.bfloat16
    i16 = mybir.dt.int16
    Alu = mybir.AluOpType
    Act = mybir.ActivationFunctionType

    TT = [a + b for a, b in zip(T0, T1)]
    TTmax = max(TT)
    W_IDX = 8 * TTmax  # int16 index columns per block (128*TT/16)

    nc = bacc.Bacc(
        None, target_bir_lowering=False, debug=False, num_swdge_queues=NQ
    )

    xt0_d = nc.dram_tensor("xt0", [HALF, F], bf16, kind="ExternalInput")
    xt1_d = nc.dram_tensor("xt1", [G - HALF, F], bf16, kind="ExternalInput")
    idx_d = nc.dram_tensor("idx", [NBLK, P, W_IDX], i16, kind="ExternalInput")
    aux_d = nc.dram_tensor("aux", [NBLK, P, 2 * TTmax], f32, kind="ExternalInput")
    wt_d = nc.dram_tensor("wt", [P, P], bf16, kind="ExternalInput")
    iota_d = nc.dram_tensor("iota", [P, P], bf16, kind="ExternalInput")
    if apply_bias:
        bias_d = nc.dram_tensor("bias", [P, P], f32, kind="ExternalInput")
    if apply_gamma:
        gamma_d = nc.dram_tensor("gamma", [P, P], f32, kind="ExternalInput")
    if apply_beta:
        beta_d = nc.dram_tensor("beta", [P, P], f32, kind="ExternalInput")
    out_d = nc.dram_tensor("out", [L, RPC, D], f32, kind="ExternalOutput")

    NCOL = NBLK * L  # one LayerNorm stat column per (block, l)

    with tile.TileContext(nc) as tc:
        with (
            tc.tile_pool(name="const", bufs=1) as constp,
            tc.tile_pool(name="auxp", bufs=16) as auxpool,
            tc.tile_pool(name="sbuild", bufs=7) as spool,
            tc.tile_pool(name="mid", bufs=3) as midpool,
            tc.tile_pool(name="store", bufs=1) as store,
            tc.tile_pool(name="outp", bufs=6) as outp,
            tc.tile_pool(name="psA", bufs=2, space="PSUM") as psA,
            tc.tile_pool(name="psT", bufs=2, space="PSUM") as psT,
            tc.tile_pool(name="psH", bufs=2, space="PSUM") as psH,
        ):
            ident = constp.tile([P, P], f32)
            make_identity(nc, ident[:])
            wt_s = constp.tile([P, P], bf16)
            nc.sync.dma_start(wt_s[:], wt_d[:])
            iota_s = constp.tile([P, P], bf16)
            nc.sync.dma_start(iota_s[:], iota_d[:])
            if apply_bias:
                bias_s = constp.tile([P, P], f32)
                nc.sync.dma_start(bias_s[:], bias_d[:])
            if apply_gamma:
                gamma_s = constp.tile([P, P], f32)
                nc.sync.dma_start(gamma_s[:], gamma_d[:])
            if apply_beta:
                beta_s = constp.tile([P, P], f32)
                nc.sync.dma_start(beta_s[:], beta_d[:])

            s_store = store.tile([P, NCOL * P], bf16)
            sum_st = store.tile([P, NCOL], f32)
            ssq_st = store.tile([P, NCOL], f32)
            junk = store.tile([P, P], f32)
            mu = store.tile([P, NCOL], f32)
            var = store.tile([P, NCOL], f32)
            ex2 = store.tile([P, NCOL], f32)
            std = store.tile([P, NCOL], f32)
            rstd = store.tile([P, NCOL], f32)
            nmr = store.tile([P, NCOL], f32)
            eps_t = store.tile([P, 1], f32)
            nc.vector.memset(eps_t[:], LN_EPS)

            def emit_stats_phase2(b0, b1):
                """LayerNorm stats for blocks [b0,b1) + normalize + store."""
                c0, c1 = b0 * L, b1 * L
                nc.vector.tensor_scalar(
                    out=mu[:, c0:c1], in0=sum_st[:, c0:c1],
                    scalar1=1.0 / D, scalar2=None, op0=Alu.mult,
                )
                # var = ssq/D - mu^2
                nc.vector.tensor_tensor(
                    out=var[:, c0:c1], in0=mu[:, c0:c1], in1=mu[:, c0:c1],
                    op=Alu.mult,
                )
                nc.vector.tensor_scalar(
                    out=var[:, c0:c1], in0=var[:, c0:c1],
                    scalar1=-1.0, scalar2=None, op0=Alu.mult,
                )
                nc.vector.tensor_scalar(
                    out=ex2[:, c0:c1], in0=ssq_st[:, c0:c1],
                    scalar1=1.0 / D, scalar2=None, op0=Alu.mult,
                )
                nc.vector.tensor_tensor(
                    out=var[:, c0:c1], in0=var[:, c0:c1], in1=ex2[:, c0:c1],
                    op=Alu.add,
                )
                nc.scalar.activation(
                    out=std[:, c0:c1], in_=var[:, c0:c1], func=Act.Sqrt,
                    bias=eps_t[:],
                )
                nc.vector.reciprocal(rstd[:, c0:c1], std[:, c0:c1])
                nc.vector.tensor_tensor(
                    out=nmr[:, c0:c1], in0=mu[:, c0:c1], in1=rstd[:, c0:c1],
                    op=Alu.mult,
                )
                nc.vector.tensor_scalar(
                    out=nmr[:, c0:c1], in0=nmr[:, c0:c1],
                    scalar1=-1.0, scalar2=None, op0=Alu.mult,
                )
                for bj in range(b0, b1):
                    rows_b = min(P, RPC - bj * P)
                    for l in range(L):
                        col = bj * L + l
                        o_t = outp.tile([P, P], f32, tag="o")
                        nc.scalar.activation(
                            out=o_t[:],
                            in_=s_store[:, col * P : (col + 1) * P],
                            func=Act.Identity,
                            scale=rstd[:, col : col + 1],
                            bias=nmr[:, col : col + 1],
                        )
                        if apply_gamma:
                            nc.vector.tensor_tensor(
                                out=o_t[:], in0=o_t[:], in1=gamma_s[:],
                                op=Alu.mult,
                            )
                        if apply_beta:
                            nc.vector.tensor_tensor(
                                out=o_t[:], in0=o_t[:], in1=beta_s[:],
                                op=Alu.add,
                            )
                        nc.sync.dma_start(
                            out_d[l, bj * P : bj * P + rows_b, :],
                            o_t[:rows_b, :],
                        )

            SPLIT = 28

            # Manual M buffers: 2 per SWDGE queue for gen/transfer overlap.
            M_bufs = [
                store.tile([P, TTmax, F], bf16, name=f"Mbuf{j}")
                for j in range(NMBUF)
            ]

            def compute_block(bi, aux_t, M):
                tt = T0[bi] + T1[bi]
                # Batched one-hot build: two stride-0-broadcast tensor ops
                # cover all tt tiles (DVE per-op overhead dominates per-tile
                # builds in-kernel).
                S_all = spool.tile([P, TTmax, P], bf16, tag="s")
                a0 = aux_t[:, 0:tt]
                rl_exp = bass.AP(
                    a0.tensor, a0.offset,
                    [a0.ap[0], [a0.ap[1][0], tt], [0, P]],
                )
                i0 = iota_s[:]
                iota_rep = bass.AP(
                    i0.tensor, i0.offset,
                    [i0.ap[0], [0, tt], [i0.ap[1][0], P]],
                )
                nc.vector.tensor_tensor(
                    out=S_all[:, :tt, :], in0=rl_exp, in1=iota_rep,
                    op=Alu.is_equal,
                )
                v0 = aux_t[:, TTmax : TTmax + tt]
                val_exp = bass.AP(
                    v0.tensor, v0.offset,
                    [v0.ap[0], [v0.ap[1][0], tt], [0, P]],
                )
                nc.vector.tensor_tensor(
                    out=S_all[:, :tt, :], in0=S_all[:, :tt, :], in1=val_exp,
                    op=Alu.mult,
                )

                # Two PSUM banks accumulate even/odd tiles independently,
                # breaking the serial accumulate chain; combined on copy-out.
                agg_pa = psA.tile([P, F], f32, tag="aggA")
                agg_pb = psA.tile([P, F], f32, tag="aggB")
                nlast_a = ((tt - 1) // 2) * 2
                nlast_b = ((tt - 2) // 2) * 2 + 1 if tt >= 2 else -1
                for t in range(tt):
                    tgt = agg_pa if t % 2 == 0 else agg_pb
                    nc.tensor.matmul(
                        tgt[:],
                        lhsT=S_all[:, t, :],
                        rhs=M[:, t, :],
                        start=(t < 2),
                        stop=(t == nlast_a or t == nlast_b),
                    )

                agg_sb = midpool.tile([P, F], f32, tag="aggsb")
                nc.scalar.copy(agg_sb[:], agg_pa[:])
                if tt >= 2:
                    nc.vector.tensor_tensor(
                        out=agg_sb[:], in0=agg_sb[:], in1=agg_pb[:],
                        op=Alu.add,
                    )
                tr_ps = psT.tile([P, F], f32, tag="tr")
                for l in range(L):
                    nc.tensor.transpose(
                        tr_ps[:, l * P : (l + 1) * P],
                        agg_sb[:, l * P : (l + 1) * P],
                        ident[:],
                    )
                aggT = midpool.tile([P, F], bf16, tag="aggT")
                nc.scalar.copy(aggT[:], tr_ps[:])

                for l in range(L):
                    col = bi * L + l
                    h_ps = psH.tile([P, P], f32, tag="h")
                    nc.tensor.matmul(
                        h_ps[:],
                        lhsT=aggT[:, l * P : (l + 1) * P],
                        rhs=wt_s[:],
                        start=True,
                        stop=True,
                    )
                    if apply_bias:
                        hb = outp.tile([P, P], f32, tag="hb")
                        nc.vector.tensor_tensor(
                            out=hb[:], in0=h_ps[:], in1=bias_s[:], op=Alu.add
                        )
                        silu_in = hb[:]
                    else:
                        silu_in = h_ps[:]
                    s_sl = s_store[:, col * P : (col + 1) * P]
                    nc.scalar.activation(
                        out=s_sl,
                        in_=silu_in,
                        func=Act.Silu,
                        accum_out=sum_st[:, col : col + 1],
                    )
                    nc.scalar.activation(
                        out=junk[:],
                        in_=s_sl,
                        func=Act.Square,
                        accum_out=ssq_st[:, col : col + 1],
                    )

            # ---- Phase 1: gather + segment-sum + linear + SiLU + moments ----
            # Blocks are processed in groups of NQ; the gather instructions
            # of a group are emitted round-robin across its blocks so the
            # Pool engine's 4-deep wait queue holds gathers of 4 DIFFERENT
            # queues/buffers instead of one block's gang (head-of-line fix).
            MAXT = 6
            for g0 in range(0, NBLK, NQ):
                group = list(range(g0, min(g0 + NQ, NBLK)))
                aux_tiles = {}
                for bi in group:
                    idx_t = auxpool.tile([P, W_IDX], i16, tag="idx")
                    aux_t = auxpool.tile([P, 2 * TTmax], f32, tag="aux")
                    nc.sync.dma_start(idx_t[:], idx_d[bi])
                    nc.sync.dma_start(aux_t[:], aux_d[bi])
                    aux_tiles[bi] = (idx_t, aux_t)

                # per-block gather windows
                parts = {}
                for bi in group:
                    t0, t1 = T0[bi], T1[bi]
                    lst = []
                    for src_d, tpre, tcnt in ((xt0_d, 0, t0), (xt1_d, t0, t1)):
                        done = 0
                        while done < tcnt:
                            step = min(MAXT, tcnt - done)
                            lst.append((src_d, tpre + done, step))
                            done += step
                    parts[bi] = lst
                rr = max(len(v) for v in parts.values())
                for pi in range(rr):
                    for bi in group:
                        if pi >= len(parts[bi]):
                            continue
                        src_d, off, step = parts[bi][pi]
                        idx_t = aux_tiles[bi][0]
                        M = M_bufs[bi % NMBUF]
                        nc.gpsimd.dma_gather(
                            M[:, off : off + step, :],
                            src_d[:],
                            idx_t[:, 8 * off : 8 * (off + step)],
                            num_idxs=step * P,
                            num_idxs_reg=step * P,
                            elem_size=F,
                            queue_num=bi % NQ,
                        )

                for bi in group:
                    compute_block(bi, aux_tiles[bi][1], M_bufs[bi % NMBUF])

                if g0 + NQ == 44:
                    emit_stats_phase2(0, 40)

            emit_stats_phase2(40, NBLK)

    nc.compile()
    return nc


def _compute_block_body():
    return """
                # Batched one-hot build: two stride-0-broadcast tensor ops
                # cover all tt tiles (DVE per-op overhead dominates per-tile
                # builds in-kernel).
                S_all = spool.tile([P, TTmax, P], bf16, tag="s")
                a0 = aux_t[:, 0:tt]
                rl_exp = bass.AP(
                    a0.tensor, a0.offset,
                    [a0.ap[0], [a0.ap[1][0], tt], [0, P]],
                )
                i0 = iota_s[:]
                iota_rep = bass.AP(
                    i0.tensor, i0.offset,
                    [i0.ap[0], [0, tt], [i0.ap[1][0], P]],
                )
                nc.vector.tensor_tensor(
                    out=S_all[:, :tt, :], in0=rl_exp, in1=iota_rep,
                    op=Alu.is_equal,
                )
                v0 = aux_t[:, TTmax : TTmax + tt]
                val_exp = bass.AP(
                    v0.tensor, v0.offset,
                    [v0.ap[0], [v0.ap[1][0], tt], [0, P]],
                )
                nc.vector.tensor_tensor(
                    out=S_all[:, :tt, :], in0=S_all[:, :tt, :], in1=val_exp,
                    op=Alu.mult,
                )

                # Two PSUM banks accumulate even/odd tiles independently,
                # breaking the serial accumulate chain; combined on copy-out.
                agg_pa = psA.tile([P, F], f32, tag="aggA")
                agg_pb = psA.tile([P, F], f32, tag="aggB")
                nlast_a = ((tt - 1) // 2) * 2
                nlast_b = ((tt - 2) // 2) * 2 + 1 if tt >= 2 else -1
                for t in range(tt):
                    tgt = agg_pa if t % 2 == 0 else agg_pb
                    nc.tensor.matmul(
                        tgt[:],
                        lhsT=S_all[:, t, :],
                        rhs=M[:, t, :],
                        start=(t < 2),
                        stop=(t == nlast_a or t == nlast_b),
                    )

                agg_sb = midpool.tile([P, F], f32, tag="aggsb")
                nc.scalar.copy(agg_sb[:], agg_pa[:])
                if tt >= 2:
                    nc.vector.tensor_tensor(
                        out=agg_sb[:], in0=agg_sb[:], in1=agg_pb[:],
                        op=Alu.add,
                    )
                tr_ps = psT.tile([P, F], f32, tag="tr")
                for l in range(L):
                    nc.tensor.transpose(
                        tr_ps[:, l * P : (l + 1) * P],
                        agg_sb[:, l * P : (l + 1) * P],
                        ident[:],
                    )
                aggT = midpool.tile([P, F], bf16, tag="aggT")
                nc.scalar.copy(aggT[:], tr_ps[:])

                for l in range(L):
                    col = bi * L + l
                    h_ps = psH.tile([P, P], f32, tag="h")
                    nc.tensor.matmul(
                        h_ps[:],
                        lhsT=aggT[:, l * P : (l + 1) * P],
                        rhs=wt_s[:],
                        start=True,
                        stop=True,
                    )
                    if apply_bias:
                        hb = outp.tile([P, P], f32, tag="hb")
                        nc.vector.tensor_tensor(
                            out=hb[:], in0=h_ps[:], in1=bias_s[:], op=Alu.add
                        )
                        silu_in = hb[:]
                    else:
                        silu_in = h_ps[:]
                    s_sl = s_store[:, col * P : (col + 1) * P]
                    nc.scalar.activation(
                        out=s_sl,
                        in_=silu_in,
                        func=Act.Silu,
                        accum_out=sum_st[:, col : col + 1],
                    )
                    nc.scalar.activation(
                        out=junk[:],
                        in_=s_sl,
                        func=Act.Square,
                        accum_out=ssq_st[:, col : col + 1],
                    )

                # Overlap the first half's LayerNorm+store under the second
                # half's phase-1 work.
                if bi == SPLIT + 2:
                    emit_stats_phase2(0, SPLIT)

            emit_stats_phase2(SPLIT, NBLK)

    nc.compile()
    return nc


def kernel(x, rows, cols, vals, W, b, gamma, beta):
    import ml_dtypes
    from concourse import bass_utils

    x = np.asarray(x, dtype=np.float32)
    rows = np.asarray(rows, dtype=np.int64)
    cols = np.asarray(cols, dtype=np.int64)
    vals = np.asarray(vals, dtype=np.float32)
    W = np.asarray(W, dtype=np.float32)
    b = np.asarray(b, dtype=np.float32)
    gamma = np.asarray(gamma, dtype=np.float32)
    beta = np.asarray(beta, dtype=np.float32)

    # ---- host-side edge routing (the "all-to-all" of the sharding) ----
    core = rows // RPC
    rloc = rows - core * RPC
    blk = rloc >> 7
    rowloc = (rloc & 127).astype(np.float32)
    chunk = (cols >= HALF).astype(np.int64)
    idxval = (cols - chunk * HALF).astype(np.int16)
    gid = core * NBLK + blk
    key = gid * 2 + chunk  # (core, block, chunk) group

    # Secondary sort by source column: gather descriptors then read HBM in
    # ascending address order within each window (row-buffer locality).
    order = np.lexsort((cols, key))
    key_s = key[order]
    counts = np.bincount(key_s, minlength=NG * 2)
    cnt = counts.reshape(N_CORES, NBLK, 2)
    T0 = [int(v) for v in np.ceil(cnt[:, :, 0].max(axis=0) / P).astype(np.int64)]
    T1 = [int(v) for v in np.ceil(cnt[:, :, 1].max(axis=0) / P).astype(np.int64)]
    TT = [a + b2 for a, b2 in zip(T0, T1)]
    TTmax = max(TT)
    W_IDX = 8 * TTmax

    starts = np.zeros(NG * 2, dtype=np.int64)
    np.cumsum(counts[:-1], out=starts[1:])
    pos = np.arange(E, dtype=np.int64) - starts[key_s]  # chunk-local slot

    core_s = core[order]
    blk_s = blk[order]
    chunk_s = chunk[order]
    T0_arr = np.asarray(T0, dtype=np.int64)
    # flat slot within the block's combined tile list
    flat = pos + chunk_s * T0_arr[blk_s] * P

    idx_plane = np.zeros((N_CORES, NBLK, 16, W_IDX), dtype=np.int16)
    idx_plane[
        core_s, blk_s, pos % 16, 8 * chunk_s * T0_arr[blk_s] + pos // 16
    ] = idxval[order]
    idx_rep = np.ascontiguousarray(np.tile(idx_plane, (1, 1, 8, 1)))

    aux = np.zeros((N_CORES, NBLK, P, 2 * TTmax), dtype=np.float32)
    aux[core_s, blk_s, flat % P, flat // P] = rowloc[order]
    aux[core_s, blk_s, flat % P, TTmax + flat // P] = vals[order]

    bf = ml_dtypes.bfloat16
    xt = np.ascontiguousarray(x.transpose(1, 0, 2).reshape(G, F).astype(bf))
    xt0 = np.ascontiguousarray(xt[:HALF])
    xt1 = np.ascontiguousarray(xt[HALF:])
    wt = np.ascontiguousarray(W.T.astype(bf))
    iota_b = np.ascontiguousarray(
        np.tile(np.arange(P, dtype=np.float32), (P, 1)).astype(bf)
    )

    apply_bias = bool(np.any(b != 0))
    apply_gamma = bool(np.any(gamma != 1))
    apply_beta = bool(np.any(beta != 0))

    key_prog = (tuple(T0), tuple(T1), apply_bias, apply_gamma, apply_beta)
    if key_prog not in _CACHE:
        _CACHE[key_prog] = _build_program(
            T0, T1, apply_bias, apply_gamma, apply_beta
        )
    nc = _CACHE[key_prog]

    in_maps = []
    for k in range(N_CORES):
        m = {
            "xt0": xt0,
            "xt1": xt1,
            "idx": idx_rep[k],
            "aux": aux[k],
            "wt": wt,
            "iota": iota_b,
        }
        if apply_bias:
            m["bias"] = np.ascontiguousarray(np.tile(b, (P, 1)))
        if apply_gamma:
            m["gamma"] = np.ascontiguousarray(np.tile(gamma, (P, 1)))
        if apply_beta:
            m["beta"] = np.ascontiguousarray(np.tile(beta, (P, 1)))
        in_maps.append(m)

    res = bass_utils.run_bass_kernel_spmd(nc, in_maps, list(range(N_CORES)))

    out = np.empty((L, G, D), dtype=np.float32)
    for k in range(N_CORES):
        out[:, k * RPC : (k + 1) * RPC, :] = res.results[k]["out"]
    return out


# revision 27
# speedup vs baseline: 1.0712x; 1.0712x over previous
"""Trainium2 Bass kernel for batched GNN message passing.

Computes, for x:[L,G,D], COO edges (rows, cols, vals), W:[D,D], b, gamma, beta:
    xt  = x.transpose(1,0,2).reshape(G, L*D)
    agg = segment_sum(xt[cols] * vals[:,None], rows, G)     # [G, L*D]
    h   = einsum('lgd,od->lgo', agg_as_lgd, W) + b
    s   = silu(h)
    out = layernorm(s) * gamma + beta                        # LN over D

Sharding: nodes (G) are split evenly across 8 NeuronCores; edges are routed
on the host to the core that owns their destination row, grouped into
128-row blocks, and padded to a uniform per-block tile count so all cores
run one SPMD program.  xt is converted to bf16 and replicated into every
core's HBM (split into two <32768-row halves because dma_gather indices are
int16), so message gathering is a local hardware dma_gather (512B rows).

The SWDGE descriptor generation for dma_gather runs on a single Q7
core-pair selected by queue_num (ucode: cpu_id/2 == queue_num), so gathers
are spread round-robin over all 4 SWDGE queues — 4 descriptor generators
run in parallel, which is the main speedup over the 1-queue baseline.

Per 128-edge tile, a one-hot selection matrix S[e,r] = vals[e]*(rowloc[e]==r)
is built on the VectorEngine (bf16) and the segment-sum becomes S.T @ M
accumulated in PSUM.  The 128x128 linear (bf16), SiLU and LayerNorm run
on-chip; PSUM->SBUF copies, the sum/sum-of-squares accumulation, and the
final normalization run on the Scalar engine to keep the VectorEngine free
for the S-tiles.
"""

import numpy as np

L, G, D, E = 2, 50000, 128, 800000
N_CORES = 8
RPC = G // N_CORES            # rows per core = 6250
P = 128
NBLK = (RPC + P - 1) // P     # 49 blocks per core (last block has 106 rows)
F = L * D                     # 256 = packed feature width of xt
NG = N_CORES * NBLK           # 392 (core, block) groups
HALF = 25000                  # xt row-split so gather indices fit in int16
LN_EPS = 1e-5
NQ = 4                        # SWDGE queues used round-robin
NMBUF = 14                    # M gather buffers (3-4 per queue)

_CACHE: dict = {}


def _build_program(T0, T1, apply_bias, apply_gamma, apply_beta):
    import concourse.bacc as bacc
    import concourse.bass as bass
    import concourse.mybir as mybir
    import concourse.tile as tile
    from concourse.masks import make_identity

    f32 = mybir.dt.float32
    bf16 = mybir.dt.bfloat16
    i16 = mybir.dt.int16
    Alu = mybir.AluOpType
    Act = mybir.ActivationFunctionType

    TT = [a + b for a, b in zip(T0, T1)]
    TTmax = max(TT)
    W_IDX = 8 * TTmax  # int16 index columns per block (128*TT/16)

    nc = bacc.Bacc(
        None, target_bir_lowering=False, debug=False, num_swdge_queues=NQ
    )

    xt0_d = nc.dram_tensor("xt0", [HALF, F], bf16, kind="ExternalInput")
    xt1_d = nc.dram_tensor("xt1", [G - HALF, F], bf16, kind="ExternalInput")
    idx_d = nc.dram_tensor("idx", [NBLK, P, W_IDX], i16, kind="ExternalInput")
    aux_d = nc.dram_tensor("aux", [NBLK, P, 2 * TTmax], f32, kind="ExternalInput")
    wt_d = nc.dram_tensor("wt", [P, P], bf16, kind="ExternalInput")
    iota_d = nc.dram_tensor("iota", [P, P], bf16, kind="ExternalInput")
    if apply_bias:
        bias_d = nc.dram_tensor("bias", [P, P], f32, kind="ExternalInput")
    if apply_gamma:
        gamma_d = nc.dram_tensor("gamma", [P, P], f32, kind="ExternalInput")
    if apply_beta:
        beta_d = nc.dram_tensor("beta", [P, P], f32, kind="ExternalInput")
    out_d = nc.dram_tensor("out", [L, RPC, D], f32, kind="ExternalOutput")

    NCOL = NBLK * L  # one LayerNorm stat column per (block, l)

    with tile.TileContext(nc) as tc:
        with (
            tc.tile_pool(name="const", bufs=1) as constp,
            tc.tile_pool(name="auxp", bufs=16) as auxpool,
            tc.tile_pool(name="sbuild", bufs=5) as spool,
            tc.tile_pool(name="mid", bufs=3) as midpool,
            tc.tile_pool(name="store", bufs=1) as store,
            tc.tile_pool(name="outp", bufs=4) as outp,
            tc.tile_pool(name="psA", bufs=2, space="PSUM") as psA,
            tc.tile_pool(name="psT", bufs=2, space="PSUM") as psT,
            tc.tile_pool(name="psH", bufs=2, space="PSUM") as psH,
        ):
            ident = constp.tile([P, P], f32)
            make_identity(nc, ident[:])
            wt_s = constp.tile([P, P], bf16)
            nc.sync.dma_start(wt_s[:], wt_d[:])
            iota_s = constp.tile([P, P], bf16)
            nc.sync.dma_start(iota_s[:], iota_d[:])
            if apply_bias:
                bias_s = constp.tile([P, P], f32)
                nc.sync.dma_start(bias_s[:], bias_d[:])
            if apply_gamma:
                gamma_s = constp.tile([P, P], f32)
                nc.sync.dma_start(gamma_s[:], gamma_d[:])
            if apply_beta:
                beta_s = constp.tile([P, P], f32)
                nc.sync.dma_start(beta_s[:], beta_d[:])

            s_store = store.tile([P, NCOL * P], bf16)
            sum_st = store.tile([P, NCOL], f32)
            ssq_st = store.tile([P, NCOL], f32)
            junk = store.tile([P, P], f32)
            mu = store.tile([P, NCOL], f32)
            var = store.tile([P, NCOL], f32)
            ex2 = store.tile([P, NCOL], f32)
            std = store.tile([P, NCOL], f32)
            rstd = store.tile([P, NCOL], f32)
            nmr = store.tile([P, NCOL], f32)
            eps_t = store.tile([P, 1], f32)
            nc.vector.memset(eps_t[:], LN_EPS)

            def emit_stats_phase2(b0, b1):
                """LayerNorm stats for blocks [b0,b1) + normalize + store."""
                c0, c1 = b0 * L, b1 * L
                nc.vector.tensor_scalar(
                    out=mu[:, c0:c1], in0=sum_st[:, c0:c1],
                    scalar1=1.0 / D, scalar2=None, op0=Alu.mult,
                )
                # var = ssq/D - mu^2
                nc.vector.tensor_tensor(
                    out=var[:, c0:c1], in0=mu[:, c0:c1], in1=mu[:, c0:c1],
                    op=Alu.mult,
                )
                nc.vector.tensor_scalar(
                    out=var[:, c0:c1], in0=var[:, c0:c1],
                    scalar1=-1.0, scalar2=None, op0=Alu.mult,
                )
                nc.vector.tensor_scalar(
                    out=ex2[:, c0:c1], in0=ssq_st[:, c0:c1],
                    scalar1=1.0 / D, scalar2=None, op0=Alu.mult,
                )
                nc.vector.tensor_tensor(
                    out=var[:, c0:c1], in0=var[:, c0:c1], in1=ex2[:, c0:c1],
                    op=Alu.add,
                )
                nc.scalar.activation(
                    out=std[:, c0:c1], in_=var[:, c0:c1], func=Act.Sqrt,
                    bias=eps_t[:],
                )
                nc.vector.reciprocal(rstd[:, c0:c1], std[:, c0:c1])
                nc.vector.tensor_tensor(
                    out=nmr[:, c0:c1], in0=mu[:, c0:c1], in1=rstd[:, c0:c1],
                    op=Alu.mult,
                )
                nc.vector.tensor_scalar(
                    out=nmr[:, c0:c1], in0=nmr[:, c0:c1],
                    scalar1=-1.0, scalar2=None, op0=Alu.mult,
                )
                for bj in range(b0, b1):
                    rows_b = min(P, RPC - bj * P)
                    for l in range(L):
                        col = bj * L + l
                        o_t = outp.tile([P, P], f32, tag="o")
                        nc.scalar.activation(
                            out=o_t[:],
                            in_=s_store[:, col * P : (col + 1) * P],
                            func=Act.Identity,
                            scale=rstd[:, col : col + 1],
                            bias=nmr[:, col : col + 1],
                        )
                        if apply_gamma:
                            nc.vector.tensor_tensor(
                                out=o_t[:], in0=o_t[:], in1=gamma_s[:],
                                op=Alu.mult,
                            )
                        if apply_beta:
                            nc.vector.tensor_tensor(
                                out=o_t[:], in0=o_t[:], in1=beta_s[:],
                                op=Alu.add,
                            )
                        nc.sync.dma_start(
                            out_d[l, bj * P : bj * P + rows_b, :],
                            o_t[:rows_b, :],
                        )

            SPLIT = 28

            # Manual M buffers: 2 per SWDGE queue for gen/transfer overlap.
            M_bufs = [
                store.tile([P, TTmax, F], bf16, name=f"Mbuf{j}")
                for j in range(NMBUF)
            ]

            def compute_block(bi, aux_t, M):
                tt = T0[bi] + T1[bi]
                # Batched one-hot build: two stride-0-broadcast tensor ops
                # cover all tt tiles (DVE per-op overhead dominates per-tile
                # builds in-kernel).
                S_all = spool.tile([P, TTmax, P], bf16, tag="s")
                a0 = aux_t[:, 0:tt]
                rl_exp = bass.AP(
                    a0.tensor, a0.offset,
                    [a0.ap[0], [a0.ap[1][0], tt], [0, P]],
                )
                i0 = iota_s[:]
                iota_rep = bass.AP(
                    i0.tensor, i0.offset,
                    [i0.ap[0], [0, tt], [i0.ap[1][0], P]],
                )
                nc.vector.tensor_tensor(
                    out=S_all[:, :tt, :], in0=rl_exp, in1=iota_rep,
                    op=Alu.is_equal,
                )
                v0 = aux_t[:, TTmax : TTmax + tt]
                val_exp = bass.AP(
                    v0.tensor, v0.offset,
                    [v0.ap[0], [v0.ap[1][0], tt], [0, P]],
                )
                nc.vector.tensor_tensor(
                    out=S_all[:, :tt, :], in0=S_all[:, :tt, :], in1=val_exp,
                    op=Alu.mult,
                )

                # Two PSUM banks accumulate even/odd tiles independently,
                # breaking the serial accumulate chain; combined on copy-out.
                agg_pa = psA.tile([P, F], f32, tag="aggA")
                agg_pb = psA.tile([P, F], f32, tag="aggB")
                nlast_a = ((tt - 1) // 2) * 2
                nlast_b = ((tt - 2) // 2) * 2 + 1 if tt >= 2 else -1
                for t in range(tt):
                    tgt = agg_pa if t % 2 == 0 else agg_pb
                    nc.tensor.matmul(
                        tgt[:],
                        lhsT=S_all[:, t, :],
                        rhs=M[:, t, :],
                        start=(t < 2),
                        stop=(t == nlast_a or t == nlast_b),
                    )

                agg_sb = midpool.tile([P, F], f32, tag="aggsb")
                nc.scalar.copy(agg_sb[:], agg_pa[:])
                if tt >= 2:
                    nc.vector.tensor_tensor(
                        out=agg_sb[:], in0=agg_sb[:], in1=agg_pb[:],
                        op=Alu.add,
                    )
                tr_ps = psT.tile([P, F], f32, tag="tr")
                for l in range(L):
                    nc.tensor.transpose(
                        tr_ps[:, l * P : (l + 1) * P],
                        agg_sb[:, l * P : (l + 1) * P],
                        ident[:],
                    )
                aggT = midpool.tile([P, F], bf16, tag="aggT")
                nc.scalar.copy(aggT[:], tr_ps[:])

                for l in range(L):
                    col = bi * L + l
                    h_ps = psH.tile([P, P], f32, tag="h")
                    nc.tensor.matmul(
                        h_ps[:],
                        lhsT=aggT[:, l * P : (l + 1) * P],
                        rhs=wt_s[:],
                        start=True,
                        stop=True,
                    )
                    if apply_bias:
                        hb = outp.tile([P, P], f32, tag="hb")
                        nc.vector.tensor_tensor(
                            out=hb[:], in0=h_ps[:], in1=bias_s[:], op=Alu.add
                        )
                        silu_in = hb[:]
                    else:
                        silu_in = h_ps[:]
                    s_sl = s_store[:, col * P : (col + 1) * P]
                    nc.scalar.activation(
                        out=s_sl,
                        in_=silu_in,
                        func=Act.Silu,
                        accum_out=sum_st[:, col : col + 1],
                    )
                    nc.scalar.activation(
                        out=junk[:],
                        in_=s_sl,
                        func=Act.Square,
                        accum_out=ssq_st[:, col : col + 1],
                    )

            # ---- Phase 1: gather + segment-sum + linear + SiLU + moments ----
            # Blocks are processed in groups of NQ; the gather instructions
            # of a group are emitted round-robin across its blocks so the
            # Pool engine's 4-deep wait queue holds gathers of 4 DIFFERENT
            # queues/buffers instead of one block's gang (head-of-line fix).
            MAXT = 6
            for g0 in range(0, NBLK, NQ):
                group = list(range(g0, min(g0 + NQ, NBLK)))
                aux_tiles = {}
                for bi in group:
                    idx_t = auxpool.tile([P, W_IDX], i16, tag="idx")
                    aux_t = auxpool.tile([P, 2 * TTmax], f32, tag="aux")
                    nc.sync.dma_start(idx_t[:], idx_d[bi])
                    nc.sync.dma_start(aux_t[:], aux_d[bi])
                    aux_tiles[bi] = (idx_t, aux_t)

                # per-block gather windows
                parts = {}
                for bi in group:
                    t0, t1 = T0[bi], T1[bi]
                    lst = []
                    for src_d, tpre, tcnt in ((xt0_d, 0, t0), (xt1_d, t0, t1)):
                        done = 0
                        while done < tcnt:
                            step = min(MAXT, tcnt - done)
                            lst.append((src_d, tpre + done, step))
                            done += step
                    parts[bi] = lst
                rr = max(len(v) for v in parts.values())
                for pi in range(rr):
                    for bi in group:
                        if pi >= len(parts[bi]):
                            continue
                        src_d, off, step = parts[bi][pi]
                        idx_t = aux_tiles[bi][0]
                        M = M_bufs[bi % NMBUF]
                        nc.gpsimd.dma_gather(
                            M[:, off : off + step, :],
                            src_d[:],
                            idx_t[:, 8 * off : 8 * (off + step)],
                            num_idxs=step * P,
                            num_idxs_reg=step * P,
                            elem_size=F,
                            queue_num=bi % NQ,
                        )

                for bi in group:
                    compute_block(bi, aux_tiles[bi][1], M_bufs[bi % NMBUF])

                if g0 + NQ == 40:
                    emit_stats_phase2(0, 36)

            emit_stats_phase2(36, NBLK)

    nc.compile()
    return nc


def _compute_block_body():
    return """
                # Batched one-hot build: two stride-0-broadcast tensor ops
                # cover all tt tiles (DVE per-op overhead dominates per-tile
                # builds in-kernel).
                S_all = spool.tile([P, TTmax, P], bf16, tag="s")
                a0 = aux_t[:, 0:tt]
                rl_exp = bass.AP(
                    a0.tensor, a0.offset,
                    [a0.ap[0], [a0.ap[1][0], tt], [0, P]],
                )
                i0 = iota_s[:]
                iota_rep = bass.AP(
                    i0.tensor, i0.offset,
                    [i0.ap[0], [0, tt], [i0.ap[1][0], P]],
                )
                nc.vector.tensor_tensor(
                    out=S_all[:, :tt, :], in0=rl_exp, in1=iota_rep,
                    op=Alu.is_equal,
                )
                v0 = aux_t[:, TTmax : TTmax + tt]
                val_exp = bass.AP(
                    v0.tensor, v0.offset,
                    [v0.ap[0], [v0.ap[1][0], tt], [0, P]],
                )
                nc.vector.tensor_tensor(
                    out=S_all[:, :tt, :], in0=S_all[:, :tt, :], in1=val_exp,
                    op=Alu.mult,
                )

                # Two PSUM banks accumulate even/odd tiles independently,
                # breaking the serial accumulate chain; combined on copy-out.
                agg_pa = psA.tile([P, F], f32, tag="aggA")
                agg_pb = psA.tile([P, F], f32, tag="aggB")
                nlast_a = ((tt - 1) // 2) * 2
                nlast_b = ((tt - 2) // 2) * 2 + 1 if tt >= 2 else -1
                for t in range(tt):
                    tgt = agg_pa if t % 2 == 0 else agg_pb
                    nc.tensor.matmul(
                        tgt[:],
                        lhsT=S_all[:, t, :],
                        rhs=M[:, t, :],
                        start=(t < 2),
                        stop=(t == nlast_a or t == nlast_b),
                    )

                agg_sb = midpool.tile([P, F], f32, tag="aggsb")
                nc.scalar.copy(agg_sb[:], agg_pa[:])
                if tt >= 2:
                    nc.vector.tensor_tensor(
                        out=agg_sb[:], in0=agg_sb[:], in1=agg_pb[:],
                        op=Alu.add,
                    )
                tr_ps = psT.tile([P, F], f32, tag="tr")
                for l in range(L):
                    nc.tensor.transpose(
                        tr_ps[:, l * P : (l + 1) * P],
                        agg_sb[:, l * P : (l + 1) * P],
                        ident[:],
                    )
                aggT = midpool.tile([P, F], bf16, tag="aggT")
                nc.scalar.copy(aggT[:], tr_ps[:])

                for l in range(L):
                    col = bi * L + l
                    h_ps = psH.tile([P, P], f32, tag="h")
                    nc.tensor.matmul(
                        h_ps[:],
                        lhsT=aggT[:, l * P : (l + 1) * P],
                        rhs=wt_s[:],
                        start=True,
                        stop=True,
                    )
                    if apply_bias:
                        hb = outp.tile([P, P], f32, tag="hb")
                        nc.vector.tensor_tensor(
                            out=hb[:], in0=h_ps[:], in1=bias_s[:], op=Alu.add
                        )
                        silu_in = hb[:]
                    else:
                        silu_in = h_ps[:]
                    s_sl = s_store[:, col * P : (col + 1) * P]
                    nc.scalar.activation(
                        out=s_sl,
                        in_=silu_in,
                        func=Act.Silu,
                        accum_out=sum_st[:, col : col + 1],
                    )
                    nc.scalar.activation(
                        out=junk[:],
                        in_=s_sl,
                        func=Act.Square,
                        accum_out=ssq_st[:, col : col + 1],
                    )

                # Overlap the first half's LayerNorm+store under the second
                # half's phase-1 work.
                if bi == SPLIT + 2:
                    emit_stats_phase2(0, SPLIT)

            emit_stats_phase2(SPLIT, NBLK)

    nc.compile()
    return nc


def kernel(x, rows, cols, vals, W, b, gamma, beta):
    import ml_dtypes
    from concourse import bass_utils

    x = np.asarray(x, dtype=np.float32)
    rows = np.asarray(rows, dtype=np.int64)
    cols = np.asarray(cols, dtype=np.int64)
    vals = np.asarray(vals, dtype=np.float32)
    W = np.asarray(W, dtype=np.float32)
    b = np.asarray(b, dtype=np.float32)
    gamma = np.asarray(gamma, dtype=np.float32)
    beta = np.asarray(beta, dtype=np.float32)

    # ---- host-side edge routing (the "all-to-all" of the sharding) ----
    core = rows // RPC
    rloc = rows - core * RPC
    blk = rloc >> 7
    rowloc = (rloc & 127).astype(np.float32)
    chunk = (cols >= HALF).astype(np.int64)
    idxval = (cols - chunk * HALF).astype(np.int16)
    gid = core * NBLK + blk
    key = gid * 2 + chunk  # (core, block, chunk) group

    # Secondary sort by source column: gather descriptors then read HBM in
    # ascending address order within each window (row-buffer locality).
    order = np.lexsort((cols, key))
    key_s = key[order]
    counts = np.bincount(key_s, minlength=NG * 2)
    cnt = counts.reshape(N_CORES, NBLK, 2)
    T0 = [int(v) for v in np.ceil(cnt[:, :, 0].max(axis=0) / P).astype(np.int64)]
    T1 = [int(v) for v in np.ceil(cnt[:, :, 1].max(axis=0) / P).astype(np.int64)]
    TT = [a + b2 for a, b2 in zip(T0, T1)]
    TTmax = max(TT)
    W_IDX = 8 * TTmax

    starts = np.zeros(NG * 2, dtype=np.int64)
    np.cumsum(counts[:-1], out=starts[1:])
    pos = np.arange(E, dtype=np.int64) - starts[key_s]  # chunk-local slot

    core_s = core[order]
    blk_s = blk[order]
    chunk_s = chunk[order]
    T0_arr = np.asarray(T0, dtype=np.int64)
    # flat slot within the block's combined tile list
    flat = pos + chunk_s * T0_arr[blk_s] * P

    idx_plane = np.zeros((N_CORES, NBLK, 16, W_IDX), dtype=np.int16)
    idx_plane[
        core_s, blk_s, pos % 16, 8 * chunk_s * T0_arr[blk_s] + pos // 16
    ] = idxval[order]
    idx_rep = np.ascontiguousarray(np.tile(idx_plane, (1, 1, 8, 1)))

    aux = np.zeros((N_CORES, NBLK, P, 2 * TTmax), dtype=np.float32)
    aux[core_s, blk_s, flat % P, flat // P] = rowloc[order]
    aux[core_s, blk_s, flat % P, TTmax + flat // P] = vals[order]

    bf = ml_dtypes.bfloat16
    xt = np.ascontiguousarray(x.transpose(1, 0, 2).reshape(G, F).astype(bf))
    xt0 = np.ascontiguousarray(xt[:HALF])
    xt1 = np.ascontiguousarray(xt[HALF:])
    wt = np.ascontiguousarray(W.T.astype(bf))
    iota_b = np.ascontiguousarray(
        np.tile(np.arange(P, dtype=np.float32), (P, 1)).astype(bf)
    )

    apply_bias = bool(np.any(b != 0))
    apply_gamma = bool(np.any(gamma != 1))
    apply_beta = bool(np.any(beta != 0))

    key_prog = (tuple(T0), tuple(T1), apply_bias, apply_gamma, apply_beta)
    if key_prog not in _CACHE:
        _CACHE[key_prog] = _build_program(
            T0, T1, apply_bias, apply_gamma, apply_beta
        )
    nc = _CACHE[key_prog]

    in_maps = []
    for k in range(N_CORES):
        m = {
            "xt0": xt0,
            "xt1": xt1,
            "idx": idx_rep[k],
            "aux": aux[k],
            "wt": wt,
            "iota": iota_b,
        }
        if apply_bias:
            m["bias"] = np.ascontiguousarray(np.tile(b, (P, 1)))
        if apply_gamma:
            m["gamma"] = np.ascontiguousarray(np.tile(gamma, (P, 1)))
        if apply_beta:
            m["beta"] = np.ascontiguousarray(np.tile(beta, (P, 1)))
        in_maps.append(m)

    res = bass_utils.run_bass_kernel_spmd(nc, in_maps, list(range(N_CORES)))

    out = np.empty((L, G, D), dtype=np.float32)
    for k in range(N_CORES):
        out[:, k * RPC : (k + 1) * RPC, :] = res.results[k]["out"]
    return out
